# revision 31
# baseline (speedup 1.0000x reference)
"""Trainium2 Bass kernel for nn_AttentiveGraph (GNN message passing).

Math reformulation (per graph, per iteration):
    att_e = exp(A[src_e] + L[dst_e] + b) = expA[src_e] * expL[dst_e]
with  A = states @ Wsa,  L = states @ Wlsa,  expL = exp(L + attention_b).
Let  U = expL * states  (elementwise).  Then with the (symmetric) edge-count
adjacency matrix  Adj[d, s] = #edges(s -> d):
    P[n] = sum_e[src=n] expL[dst_e]  =  (Adj @ expL)[n]
    Q[n] = sum_e[src=n] (expL*states)[dst_e] = (Adj @ U)[n]
    norm = 1 + expA * P ;  r = 1/norm
    linked_gated = (expA * Q) * r
    states' = tanh(states * r + linked_gated @ Wl + state_b)

So the whole gather/scatter edge phase becomes a dense SpMM with a
compile-time-constant 0/1/2-valued adjacency, done on the PE:
    out[c, s] += sum_d M_j[d, c] * Adj[d, s]      (c-major everywhere)
stationary = M_j (expL/U node-major tiles, bf16), moving = Adj (fp8, exact).

Distribution: 8 cores = 4 graphs x 2 src-half splits. Node indices are
CORE-LOCAL ordered ([own half || peer half], encoded host-side in each
core's adjacency layout), so each core's own half needs no exchange. The
peer half is exchanged with a pairwise AllReduce-ADD of own halves; each
core recovers the peer's data as (sum - own). This keeps the program
identical across cores (pure SPMD, no rank-dependent offsets) and lets the
next iteration's own-half node phase start while the collective runs.
"""

import numpy as np

import concourse.bass as bass
import concourse.mybir as mybir
import concourse.tile as tile
from concourse import bacc
from concourse.bass_utils import run_bass_kernel_spmd
from concourse.masks import make_identity

F32 = mybir.dt.float32
F32R = mybir.dt.float32r
BF16 = mybir.dt.bfloat16
FP8 = mybir.dt.float8e4

B, N, E, F, C = 4, 10000, 160000, 128, 128
NUM_ITERATIONS = 3
P = 128          # partitions
CH = 512         # free-dim chunk (one PSUM bank of f32)


class Cfg:
    def __init__(self, npad, group=2, jg=8, iters=NUM_ITERATIONS,
                 node_bf16=False, fp8_p=False, fp8_q=False, al_bf16=True,
                 hl_fp8=False):
        self.node_bf16 = node_bf16
        self.hl_fp8 = hl_fp8
        self.al_bf16 = al_bf16
        self.fp8_p = fp8_p
        self.fp8_q = fp8_q
        assert npad % (2 * CH) == 0 and (npad // P) % jg == 0
        self.npad = npad
        self.nt = npad // P            # dst tiles (contraction)
        self.half = npad // 2          # src nodes per core
        self.hchunks = self.half // CH  # 512-chunks per half
        self.group = group             # chunks processed per stationary sweep
        assert self.hchunks % group == 0
        self.ngroups = self.hchunks // group
        self.jg = jg                   # dst tiles per adjacency DMA
        self.iters = iters


def build_kernel(cfg: Cfg, no_collective=False):
    """Build the single-program SPMD kernel (same program on all 8 cores).

    no_collective=True replaces the AllReduce with a local DMA (wrong data,
    same cost shape) so the kernel can run under single-core TimelineSim.
    """
    nc = bacc.Bacc("TRN2", target_bir_lowering=False, num_devices=8)
    npad, nt, half, hch = cfg.npad, cfg.nt, cfg.half, cfg.hchunks
    G, JG = cfg.group, cfg.jg
    fchunks = npad // CH               # full-range 512-chunks

    # ---- DRAM I/O ----
    assert nt % 2 == 0
    adj = nc.dram_tensor("adj", [cfg.ngroups, P, nt // 2, 2, G * CH], FP8,
                         kind="ExternalInput")
    objT_full = nc.dram_tensor("objT_full", [P, npad], F32, kind="ExternalInput")
    w_os = nc.dram_tensor("w_os", [F, C], F32, kind="ExternalInput")
    w_sa = nc.dram_tensor("w_sa", [C, C], BF16, kind="ExternalInput")
    w_lsa = nc.dram_tensor("w_lsa", [C, C], BF16, kind="ExternalInput")
    w_l = nc.dram_tensor("w_l", [C, C], BF16, kind="ExternalInput")
    att_b = nc.dram_tensor("att_b", [C, 1], F32, kind="ExternalInput")
    st_b = nc.dram_tensor("st_b", [C, 1], F32, kind="ExternalInput")
    w_sa_f = nc.dram_tensor("w_sa_f", [C, C], F32, kind="ExternalInput")
    w_lsa_f = nc.dram_tensor("w_lsa_f", [C, C], F32, kind="ExternalInput")
    w_l_f = nc.dram_tensor("w_l_f", [C, C], F32, kind="ExternalInput")
    out_states = nc.dram_tensor("out_states", [P, half], F32,
                                kind="ExternalOutput")

    groups = [[0, 1], [2, 3], [4, 5], [6, 7]]
    Exp = mybir.ActivationFunctionType.Exp
    Tanh = mybir.ActivationFunctionType.Tanh

    with tile.TileContext(nc) as tc:
        with (
            tc.tile_pool(name="const", bufs=1) as const,
            tc.tile_pool(name="persist", bufs=1) as persist,
            tc.tile_pool(name="dram", bufs=3, space="DRAM") as dram,
            tc.tile_pool(name="psA", bufs=1, space="PSUM") as psA,
            tc.tile_pool(name="psB", bufs=2, space="PSUM") as psB,
            tc.tile_pool(name="psT", bufs=1, space="PSUM") as psT,
        ):
            # constants
            pdt = FP8 if (cfg.fp8_p or cfg.hl_fp8) else BF16
            qdt = FP8 if (cfg.fp8_q or cfg.hl_fp8) else BF16
            ident_p = const.tile([P, P], pdt)
            make_identity(nc, ident_p[:])
            ident_bfT = const.tile([P, P], BF16)
            make_identity(nc, ident_bfT[:])
            if qdt == pdt:
                ident_q = ident_p
            else:
                ident_q = const.tile([P, P], qdt)
                make_identity(nc, ident_q[:])
            wos_t = const.tile([F, C], F32)
            nc.sync.dma_start(wos_t[:], w_os[:])
            wsa_t = const.tile([C, C], BF16)
            nc.sync.dma_start(wsa_t[:], w_sa[:])
            wlsa_t = const.tile([C, C], BF16)
            nc.sync.dma_start(wlsa_t[:], w_lsa[:])
            wl_t = const.tile([C, C], BF16)
            nc.sync.dma_start(wl_t[:], w_l[:])
            attb_t = const.tile([C, 1], F32)
            nc.sync.dma_start(attb_t[:], att_b[:])
            stb_t = const.tile([C, 1], F32)
            nc.sync.dma_start(stb_t[:], st_b[:])
            if not cfg.node_bf16:
                wsa_f = const.tile([C, C], F32)
                nc.sync.dma_start(wsa_f[:], w_sa_f[:])
                wlsa_f = const.tile([C, C], F32)
                nc.sync.dma_start(wlsa_f[:], w_lsa_f[:])
                wl_f = const.tile([C, C], F32)
                nc.sync.dma_start(wl_f[:], w_l_f[:])

            # persistent state (all node indexing is core-local:
            # [own half || peer half])
            states_cm = persist.tile([P, npad], F32)    # full, c-major
            states_bf = persist.tile([P, npad], BF16)   # matmul moving copy
            expA_own = persist.tile([P, half], F32)
            swid = 2 * P if cfg.hl_fp8 else P   # hi+lo pair per tile if hl
            expLn = persist.tile([P, nt * swid], pdt)   # node-major, per j tile
            Un = persist.tile([P, nt * swid], qdt)

            # ---- initial states ----
            with tc.tile_pool(name="init", bufs=1) as initp:
                oT = initp.tile([P, npad], F32)
                for q in range(8):
                    qb = npad // 8
                    nc.sync.dma_start(oT[:, q * qb:(q + 1) * qb],
                                      objT_full[:, q * qb:(q + 1) * qb])
                for k in range(fchunks):
                    ps = psB.tile([P, CH], F32, space="PSUM")
                    nc.tensor.matmul(ps[:], wos_t[:], oT[:, k * CH:(k + 1) * CH])
                    nc.scalar.activation(states_cm[:, k * CH:(k + 1) * CH], ps[:],
                                         Tanh, bias=stb_t[:, :1])
                    nc.vector.tensor_copy(states_bf[:, k * CH:(k + 1) * CH],
                                          states_cm[:, k * CH:(k + 1) * CH])

            # ---- iterations ----
            with (
                tc.tile_pool(name="adjp", bufs=3) as adjp,
                tc.tile_pool(name="cmtmp", bufs=3) as cmtmp,
                tc.tile_pool(name="upd", bufs=2) as upd,
                tc.tile_pool(name="snewp", bufs=3) as snewp,
                tc.tile_pool(name="sump", bufs=2) as sump,
            ):
                for it in range(cfg.iters):
                    last = it == cfg.iters - 1
                    if not last:
                        inb = dram.tile([P, half], F32, tag="inb")
                        outb = dram.tile([P, half], F32, tag="outb")
                    # expA for own half (local chunks 0..hch-1)
                    for k in range(hch):
                        sl = slice(k * CH, (k + 1) * CH)
                        ps = psB.tile([P, CH], F32, space="PSUM")
                        if cfg.node_bf16 or cfg.al_bf16:
                            nc.tensor.matmul(ps[:], wsa_t[:], states_bf[:, sl])
                        else:
                            nc.tensor.matmul(ps[:], wsa_f[:], states_cm[:, sl])
                        nc.scalar.activation(expA_own[:, sl], ps[:], Exp)
                    # node phase: expL (c-major) -> transpose -> expLn / Un
                    for k in range(fchunks):
                        sl = slice(k * CH, (k + 1) * CH)
                        ps = psB.tile([P, CH], F32, space="PSUM")
                        if cfg.node_bf16 or cfg.al_bf16:
                            nc.tensor.matmul(ps[:], wlsa_t[:], states_bf[:, sl])
                        else:
                            nc.tensor.matmul(ps[:], wlsa_f[:], states_cm[:, sl])
                        eL = cmtmp.tile([P, CH], BF16 if cfg.hl_fp8 else pdt)
                        nc.scalar.activation(eL[:], ps[:], Exp, bias=attb_t[:, :1])
                        uC = cmtmp.tile([P, CH], BF16 if cfg.hl_fp8 else qdt)
                        nc.vector.tensor_mul(uC[:], eL[:], states_bf[:, sl])
                        if cfg.hl_fp8:
                            # split each bf16 value into hi+lo fp8 (residual
                            # rounding ~0.2% = bf16-level) for DoubleRow
                            # hi = fp8(v) (deterministic rounding, applied
                            # again at eviction); lo = v - hi kept bf16 and
                            # rounded to fp8 by the eviction cast
                            hiL = cmtmp.tile([P, CH], FP8, tag="hiL", name="hiL")
                            nc.scalar.copy(hiL[:], eL[:])
                            loL = cmtmp.tile([P, CH], BF16, tag="loL", name="loL")
                            nc.vector.tensor_tensor(
                                out=loL[:], in0=eL[:], in1=hiL[:],
                                op=mybir.AluOpType.subtract)
                            hiU = cmtmp.tile([P, CH], FP8, tag="hiU", name="hiU")
                            nc.scalar.copy(hiU[:], uC[:])
                            loU = cmtmp.tile([P, CH], BF16, tag="loU", name="loU")
                            nc.vector.tensor_tensor(
                                out=loU[:], in0=uC[:], in1=hiU[:],
                                op=mybir.AluOpType.subtract)
                            pt = psT.tile([P, CH // P, 2, P], BF16, space="PSUM")
                            pt2 = psT.tile([P, CH // P, 2, P], BF16, space="PSUM",
                                           name="pt2", tag="pt2")
                            idb = ident_q if qdt == BF16 else ident_p
                            for t in range(CH // P):
                                ts_ = slice(t * P, (t + 1) * P)
                                nc.tensor.transpose(pt[:, t, 0], eL[:, ts_],
                                                    ident_bfT[:])
                                nc.tensor.transpose(pt[:, t, 1], loL[:, ts_],
                                                    ident_bfT[:])
                                nc.tensor.transpose(pt2[:, t, 0], uC[:, ts_],
                                                    ident_bfT[:])
                                nc.tensor.transpose(pt2[:, t, 1], loU[:, ts_],
                                                    ident_bfT[:])
                            hsl = slice(k * 2 * CH, (k + 1) * 2 * CH)
                            nc.vector.tensor_copy(expLn[:, hsl], pt[:])
                            nc.vector.tensor_copy(Un[:, hsl], pt2[:])
                        else:
                            pt = psT.tile([P, CH // P, P], pdt, space="PSUM")
                            pt2 = psT.tile([P, CH // P, P], qdt, space="PSUM",
                                           name="pt2", tag="pt2")
                            for t in range(CH // P):
                                nc.tensor.transpose(
                                    pt[:, t], eL[:, t * P:(t + 1) * P], ident_p[:])
                                nc.tensor.transpose(
                                    pt2[:, t], uC[:, t * P:(t + 1) * P], ident_q[:])
                            nc.vector.tensor_copy(expLn[:, sl], pt[:])
                            nc.vector.tensor_copy(Un[:, sl], pt2[:])

                    # SpMM + per-chunk update
                    for g in range(cfg.ngroups):
                        Pt = psA.tile([P, G * CH], F32, space="PSUM", tag="p")
                        Qt = psA.tile([P, G * CH], F32, space="PSUM", tag="q")
                        JP = JG // 2    # dst-tile pairs per DMA slab
                        for jg in range(nt // JG):
                            at = adjp.tile([P, JP, 2, G * CH], FP8)
                            nc.sync.dma_start(
                                at[:], adj[g, :, jg * JP:(jg + 1) * JP, :, :])
                            for jl in range(JP):
                                jp = jg * JP + jl
                                st = (jp == 0)
                                sp = (jp == nt // 2 - 1)
                                if cfg.hl_fp8:
                                    for h in range(2):
                                        j = jp * 2 + h
                                        for sta, out in ((expLn, Pt), (Un, Qt)):
                                            lhs3 = sta[:, j * 2 * P:(j + 1) * 2 * P] \
                                                .rearrange("p (two c) -> p two c",
                                                           two=2)
                                            for i in range(G):
                                                rhs3 = at[:, jl, h,
                                                          i * CH:(i + 1) * CH] \
                                                    .rearrange(
                                                        "p (one s) -> p one s",
                                                        one=1) \
                                                    .to_broadcast([P, 2, CH])
                                                nc.tensor.matmul(
                                                    out[:, i * CH:(i + 1) * CH],
                                                    lhs3, rhs3,
                                                    start=st and h == 0,
                                                    stop=sp and h == 1,
                                                    perf_mode=mybir
                                                    .MatmulPerfMode.DoubleRow)
                                    continue
                                for sta, out, fp8m in ((expLn, Pt, cfg.fp8_p),
                                                       (Un, Qt, cfg.fp8_q)):
                                    if fp8m:
                                        lhs3 = sta[:, jp * 2 * P:(jp + 1) * 2 * P] \
                                            .rearrange("p (two c) -> p two c", two=2)
                                        for i in range(G):
                                            nc.tensor.matmul(
                                                out[:, i * CH:(i + 1) * CH],
                                                lhs3,
                                                at[:, jl, :, i * CH:(i + 1) * CH],
                                                start=st, stop=sp,
                                                perf_mode=mybir.MatmulPerfMode.DoubleRow)
                                    else:
                                        for h in range(2):
                                            j = jp * 2 + h
                                            for i in range(G):
                                                nc.tensor.matmul(
                                                    out[:, i * CH:(i + 1) * CH],
                                                    sta[:, j * P:(j + 1) * P],
                                                    at[:, jl, h, i * CH:(i + 1) * CH],
                                                    start=st and h == 0,
                                                    stop=sp and h == 1)
                        norms, lgfs = [], []
                        for i in range(G):
                            k = g * G + i
                            sl = slice(k * CH, (k + 1) * CH)
                            ii = slice(i * CH, (i + 1) * CH)
                            norm = upd.tile([P, CH], F32, tag=f"norm{i}",
                                            name=f"norm{i}")
                            nc.vector.tensor_mul(norm[:], expA_own[:, sl], Pt[:, ii])
                            lgf = upd.tile([P, CH], F32, tag=f"lgf{i}",
                                           name=f"lgf{i}")
                            nc.vector.tensor_mul(lgf[:], expA_own[:, sl], Qt[:, ii])
                            norms.append(norm)
                            lgfs.append(lgf)
                        for i in range(G):
                            k = g * G + i
                            sl = slice(k * CH, (k + 1) * CH)
                            norm, lgf = norms[i], lgfs[i]
                            nc.vector.tensor_scalar_add(norm[:], norm[:], 1.0)
                            r = upd.tile([P, CH], F32, tag="r")
                            nc.vector.reciprocal_approx_fast(r[:], norm[:])
                            lg = upd.tile([P, CH],
                                          BF16 if cfg.node_bf16 else F32,
                                          tag="lg")
                            nc.vector.tensor_mul(lg[:], lgf[:], r[:])
                            ps = psB.tile([P, CH], F32, space="PSUM")
                            nc.tensor.matmul(
                                ps[:], wl_t[:] if cfg.node_bf16 else wl_f[:],
                                lg[:])
                            sv = upd.tile([P, CH], F32, tag="sv")
                            nc.vector.tensor_mul(sv[:], states_cm[:, sl], r[:])
                            nc.vector.tensor_add(sv[:], sv[:], ps[:])
                            snew = snewp.tile([P, CH], F32)
                            nc.scalar.activation(snew[:], sv[:], Tanh,
                                                 bias=stb_t[:, :1])
                            if last:
                                nc.sync.dma_start(out_states[:, sl], snew[:])
                            else:
                                nc.vector.tensor_copy(states_cm[:, sl], snew[:])
                                nc.vector.tensor_copy(states_bf[:, sl], snew[:])
                                nc.sync.dma_start(inb[:, sl], snew[:])
                    # exchange: AllReduce-ADD own halves, peer = sum - own
                    if not last:
                        if no_collective:
                            nc.sync.dma_start(outb[:], inb[:])
                        else:
                            nc.gpsimd.collective_compute(
                                "AllReduce", mybir.AluOpType.add,
                                replica_groups=groups,
                                ins=[inb.opt()], outs=[outb.opt()])
                        gb = half // 4
                        for q in range(4):
                            qs = slice(q * gb, (q + 1) * gb)
                            sm = sump.tile([P, gb], F32)
                            nc.sync.dma_start(sm[:], outb[:, qs])
                            nc.vector.tensor_tensor(
                                out=states_cm[:, half + q * gb:half + (q + 1) * gb],
                                in0=sm[:], in1=states_cm[:, qs],
                                op=mybir.AluOpType.subtract)
                            nc.vector.tensor_copy(
                                states_bf[:, half + q * gb:half + (q + 1) * gb],
                                states_cm[:, half + q * gb:half + (q + 1) * gb])

    nc.compile()
    return nc


def preprocess(inputs, cfg: Cfg):
    """FULL numpy inputs -> per-core in_maps (index/layout work only)."""
    npad, half = cfg.npad, cfg.half
    fp8_np = mybir.dt.np(FP8)
    conn = np.asarray(inputs["connections"])
    objs = np.asarray(inputs["objects"], dtype=np.float32)
    nreal = objs.shape[1]
    bf = mybir.dt.np(BF16)
    w_sa_f32 = np.asarray(inputs["state_attention_W"], np.float32)
    w_l_f32 = np.asarray(inputs["linked_state_W"], np.float32)
    w_lsa_f32 = np.asarray(inputs["linked_state_attention_W"], np.float32)
    w_os = np.asarray(inputs["object_state_W"], np.float32)
    w_sa = np.asarray(inputs["state_attention_W"], np.float32).astype(bf)
    w_lsa = np.asarray(inputs["linked_state_attention_W"], np.float32).astype(bf)
    w_l = np.asarray(inputs["linked_state_W"], np.float32).astype(bf)
    att_b = np.asarray(inputs["attention_b"], np.float32).reshape(C, 1)
    st_b = np.asarray(inputs["state_b"], np.float32).reshape(C, 1)

    in_maps = []
    for b in range(B):
        a, d = conn[b, :, 0].astype(np.int64), conn[b, :, 1].astype(np.int64)
        adjm = np.zeros((npad, npad), np.uint8)
        np.add.at(adjm, (a, d), 1)
        np.add.at(adjm, (d, a), 1)
        objT = np.zeros((F, npad), np.float32)
        objT[:, :nreal] = objs[b].T
        for hi in range(2):
            off = hi * half
            oth = (1 - hi) * half
            # core-local dst (row) order: [own half rows || peer half rows];
            # src (column) range: own half
            sub = np.concatenate(
                [adjm[off:off + half, off:off + half],
                 adjm[oth:oth + half, off:off + half]], axis=0)
            sub = sub.reshape(cfg.nt // 2, 2, P, cfg.ngroups, cfg.group * CH)
            adj_in = np.ascontiguousarray(
                sub.transpose(3, 2, 0, 1, 4)).astype(fp8_np)    # [g, d, jp, 2, s]
            objT_local = np.concatenate(
                [objT[:, off:off + half], objT[:, oth:oth + half]], axis=1)
            in_maps.append({
                "adj": adj_in,
                "objT_full": np.ascontiguousarray(objT_local),
                "w_os": w_os, "w_sa": w_sa, "w_lsa": w_lsa, "w_l": w_l,
                "w_sa_f": w_sa_f32, "w_lsa_f": w_lsa_f32, "w_l_f": w_l_f32,
                "att_b": att_b, "st_b": st_b,
            })
    return in_maps


def postprocess(results, cfg: Cfg, nreal=N):
    out = np.zeros((B, nreal, C), np.float32)
    for b in range(B):
        full = np.concatenate(
            [results[2 * b]["out_states"], results[2 * b + 1]["out_states"]],
            axis=1)
        out[b] = full[:, :nreal].T
    return out


_CACHE = {}


def kernel(**inputs) -> np.ndarray:
    cfg = Cfg(npad=10240)
    if "nc" not in _CACHE:
        _CACHE["nc"] = build_kernel(cfg)
    nc = _CACHE["nc"]
    in_maps = preprocess(inputs, cfg)
    res = run_bass_kernel_spmd(nc, in_maps, core_ids=list(range(8)))
    return postprocess(res.results, cfg)


if __name__ == "__main__":
    import reference as R
    inputs = {k: np.asarray(v) for k, v in R.setup_inputs().items()}
    got = kernel(**inputs)
    exp = np.asarray(R.reference(**R.setup_inputs()))
    am = np.abs(exp).max()
    err = np.abs(got - exp).max()
    print(f"maxabs={am:.4f} err={err:.3e} rel={err / am:.3e}")


# revision 33
# speedup vs baseline: 1.0462x; 1.0462x over previous
"""Trainium2 Bass kernel for nn_AttentiveGraph (GNN message passing).

Math reformulation (per graph, per iteration):
    att_e = exp(A[src_e] + L[dst_e] + b) = expA[src_e] * expL[dst_e]
with  A = states @ Wsa,  L = states @ Wlsa,  expL = exp(L + attention_b).
Let  U = expL * states  (elementwise).  Then with the (symmetric) edge-count
adjacency matrix  Adj[d, s] = #edges(s -> d):
    P[n] = sum_e[src=n] expL[dst_e]  =  (Adj @ expL)[n]
    Q[n] = sum_e[src=n] (expL*states)[dst_e] = (Adj @ U)[n]
    norm = 1 + expA * P ;  r = 1/norm
    linked_gated = (expA * Q) * r
    states' = tanh(states * r + linked_gated @ Wl + state_b)

So the whole gather/scatter edge phase becomes a dense SpMM with a
compile-time-constant 0/1/2-valued adjacency, done on the PE:
    out[c, s] += sum_d M_j[d, c] * Adj[d, s]      (c-major everywhere)
stationary = M_j (expL/U node-major tiles, bf16), moving = Adj (fp8, exact).

Distribution: 8 cores = 4 graphs x 2 src-half splits. Node indices are
CORE-LOCAL ordered ([own half || peer half], encoded host-side in each
core's adjacency layout), so each core's own half needs no exchange. The
peer half is exchanged with a pairwise AllReduce-ADD of own halves; each
core recovers the peer's data as (sum - own). This keeps the program
identical across cores (pure SPMD, no rank-dependent offsets) and lets the
next iteration's own-half node phase start while the collective runs.
"""

import numpy as np

import concourse.bass as bass
import concourse.mybir as mybir
import concourse.tile as tile
from concourse import bacc
from concourse.bass_utils import run_bass_kernel_spmd
from concourse.masks import make_identity

F32 = mybir.dt.float32
F32R = mybir.dt.float32r
BF16 = mybir.dt.bfloat16
FP8 = mybir.dt.float8e4

B, N, E, F, C = 4, 10000, 160000, 128, 128
NUM_ITERATIONS = 3
P = 128          # partitions
CH = 512         # free-dim chunk (one PSUM bank of f32)


class Cfg:
    def __init__(self, npad, group=1, jg=8, iters=NUM_ITERATIONS,
                 node_bf16=False, fp8_p=False, fp8_q=False, al_bf16=True,
                 hl_fp8=False):
        self.node_bf16 = node_bf16
        self.hl_fp8 = hl_fp8
        self.al_bf16 = al_bf16
        self.fp8_p = fp8_p
        self.fp8_q = fp8_q
        assert npad % (2 * CH) == 0 and (npad // P) % jg == 0
        self.npad = npad
        self.nt = npad // P            # dst tiles (contraction)
        self.half = npad // 2          # src nodes per core
        self.hchunks = self.half // CH  # 512-chunks per half
        self.group = group             # chunks processed per stationary sweep
        assert self.hchunks % group == 0
        self.ngroups = self.hchunks // group
        self.jg = jg                   # dst tiles per adjacency DMA
        self.iters = iters


def build_kernel(cfg: Cfg, no_collective=False):
    """Build the single-program SPMD kernel (same program on all 8 cores).

    no_collective=True replaces the AllReduce with a local DMA (wrong data,
    same cost shape) so the kernel can run under single-core TimelineSim.
    """
    nc = bacc.Bacc("TRN2", target_bir_lowering=False, num_devices=8)
    npad, nt, half, hch = cfg.npad, cfg.nt, cfg.half, cfg.hchunks
    G, JG = cfg.group, cfg.jg
    fchunks = npad // CH               # full-range 512-chunks

    # ---- DRAM I/O ----
    assert nt % 2 == 0
    adj = nc.dram_tensor("adj", [cfg.ngroups, P, nt // 2, 2, G * CH], FP8,
                         kind="ExternalInput")
    objT_full = nc.dram_tensor("objT_full", [P, npad], F32, kind="ExternalInput")
    w_os = nc.dram_tensor("w_os", [F, C], F32, kind="ExternalInput")
    w_sa = nc.dram_tensor("w_sa", [C, C], BF16, kind="ExternalInput")
    w_lsa = nc.dram_tensor("w_lsa", [C, C], BF16, kind="ExternalInput")
    w_l = nc.dram_tensor("w_l", [C, C], BF16, kind="ExternalInput")
    att_b = nc.dram_tensor("att_b", [C, 1], F32, kind="ExternalInput")
    st_b = nc.dram_tensor("st_b", [C, 1], F32, kind="ExternalInput")
    w_sa_f = nc.dram_tensor("w_sa_f", [C, C], F32, kind="ExternalInput")
    w_lsa_f = nc.dram_tensor("w_lsa_f", [C, C], F32, kind="ExternalInput")
    w_l_f = nc.dram_tensor("w_l_f", [C, C], F32, kind="ExternalInput")
    out_states = nc.dram_tensor("out_states", [P, half], F32,
                                kind="ExternalOutput")

    groups = [[0, 1], [2, 3], [4, 5], [6, 7]]
    Exp = mybir.ActivationFunctionType.Exp
    Tanh = mybir.ActivationFunctionType.Tanh

    with tile.TileContext(nc) as tc:
        with (
            tc.tile_pool(name="const", bufs=1) as const,
            tc.tile_pool(name="persist", bufs=1) as persist,
            tc.tile_pool(name="dram", bufs=3, space="DRAM") as dram,
            tc.tile_pool(name="psA", bufs=2 if cfg.group == 1 else 1,
                         space="PSUM") as psA,
            tc.tile_pool(name="psB", bufs=2, space="PSUM") as psB,
            tc.tile_pool(name="psT", bufs=1, space="PSUM") as psT,
        ):
            # constants
            pdt = FP8 if (cfg.fp8_p or cfg.hl_fp8) else BF16
            qdt = FP8 if (cfg.fp8_q or cfg.hl_fp8) else BF16
            ident_p = const.tile([P, P], pdt)
            make_identity(nc, ident_p[:])
            ident_bfT = const.tile([P, P], BF16)
            make_identity(nc, ident_bfT[:])
            if qdt == pdt:
                ident_q = ident_p
            else:
                ident_q = const.tile([P, P], qdt)
                make_identity(nc, ident_q[:])
            wos_t = const.tile([F, C], F32)
            nc.sync.dma_start(wos_t[:], w_os[:])
            wsa_t = const.tile([C, C], BF16)
            nc.sync.dma_start(wsa_t[:], w_sa[:])
            wlsa_t = const.tile([C, C], BF16)
            nc.sync.dma_start(wlsa_t[:], w_lsa[:])
            wl_t = const.tile([C, C], BF16)
            nc.sync.dma_start(wl_t[:], w_l[:])
            attb_t = const.tile([C, 1], F32)
            nc.sync.dma_start(attb_t[:], att_b[:])
            stb_t = const.tile([C, 1], F32)
            nc.sync.dma_start(stb_t[:], st_b[:])
            if not cfg.node_bf16:
                wsa_f = const.tile([C, C], F32)
                nc.sync.dma_start(wsa_f[:], w_sa_f[:])
                wlsa_f = const.tile([C, C], F32)
                nc.sync.dma_start(wlsa_f[:], w_lsa_f[:])
                wl_f = const.tile([C, C], F32)
                nc.sync.dma_start(wl_f[:], w_l_f[:])

            # persistent state (all node indexing is core-local:
            # [own half || peer half])
            states_cm = persist.tile([P, npad], F32)    # full, c-major
            states_bf = persist.tile([P, npad], BF16)   # matmul moving copy
            expA_own = persist.tile([P, half], F32)
            swid = 2 * P if cfg.hl_fp8 else P   # hi+lo pair per tile if hl
            expLn = persist.tile([P, nt * swid], pdt)   # node-major, per j tile
            Un = persist.tile([P, nt * swid], qdt)

            # ---- initial states ----
            with tc.tile_pool(name="init", bufs=1) as initp:
                oT = initp.tile([P, npad], F32)
                qb = npad // 32
                for q in range(4):
                    nc.sync.dma_start(oT[:, q * qb:(q + 1) * qb],
                                      objT_full[:, q * qb:(q + 1) * qb])
                qb = npad // 8
                for q in range(1, 8):
                    nc.sync.dma_start(oT[:, q * qb:(q + 1) * qb],
                                      objT_full[:, q * qb:(q + 1) * qb])
                for k in range(fchunks):
                    ps = psB.tile([P, CH], F32, space="PSUM")
                    nc.tensor.matmul(ps[:], wos_t[:], oT[:, k * CH:(k + 1) * CH])
                    nc.scalar.activation(states_cm[:, k * CH:(k + 1) * CH], ps[:],
                                         Tanh, bias=stb_t[:, :1])
                    nc.vector.tensor_copy(states_bf[:, k * CH:(k + 1) * CH],
                                          states_cm[:, k * CH:(k + 1) * CH])

            # ---- iterations ----
            with (
                tc.tile_pool(name="adjp", bufs=3) as adjp,
                tc.tile_pool(name="cmtmp", bufs=3) as cmtmp,
                tc.tile_pool(name="upd", bufs=2) as upd,
                tc.tile_pool(name="snewp", bufs=3) as snewp,
                tc.tile_pool(name="sump", bufs=2) as sump,
            ):
                for it in range(cfg.iters):
                    last = it == cfg.iters - 1
                    if not last:
                        inb = dram.tile([P, half], F32, tag="inb")
                        outb = dram.tile([P, half], F32, tag="outb")
                    # expA for own half (local chunks 0..hch-1)
                    for k in range(hch):
                        sl = slice(k * CH, (k + 1) * CH)
                        ps = psB.tile([P, CH], F32, space="PSUM")
                        if cfg.node_bf16 or cfg.al_bf16:
                            nc.tensor.matmul(ps[:], wsa_t[:], states_bf[:, sl])
                        else:
                            nc.tensor.matmul(ps[:], wsa_f[:], states_cm[:, sl])
                        nc.scalar.activation(expA_own[:, sl], ps[:], Exp)
                    # node phase: expL (c-major) -> transpose -> expLn / Un
                    for k in range(fchunks):
                        sl = slice(k * CH, (k + 1) * CH)
                        ps = psB.tile([P, CH], F32, space="PSUM")
                        if cfg.node_bf16 or cfg.al_bf16:
                            nc.tensor.matmul(ps[:], wlsa_t[:], states_bf[:, sl])
                        else:
                            nc.tensor.matmul(ps[:], wlsa_f[:], states_cm[:, sl])
                        eL = cmtmp.tile([P, CH], BF16 if cfg.hl_fp8 else pdt)
                        nc.scalar.activation(eL[:], ps[:], Exp, bias=attb_t[:, :1])
                        uC = cmtmp.tile([P, CH], BF16 if cfg.hl_fp8 else qdt)
                        nc.vector.tensor_mul(uC[:], eL[:], states_bf[:, sl])
                        if cfg.hl_fp8:
                            # split each bf16 value into hi+lo fp8 (residual
                            # rounding ~0.2% = bf16-level) for DoubleRow
                            # hi = fp8(v) (deterministic rounding, applied
                            # again at eviction); lo = v - hi kept bf16 and
                            # rounded to fp8 by the eviction cast
                            hiL = cmtmp.tile([P, CH], FP8, tag="hiL", name="hiL")
                            nc.scalar.copy(hiL[:], eL[:])
                            loL = cmtmp.tile([P, CH], BF16, tag="loL", name="loL")
                            nc.vector.tensor_tensor(
                                out=loL[:], in0=eL[:], in1=hiL[:],
                                op=mybir.AluOpType.subtract)
                            hiU = cmtmp.tile([P, CH], FP8, tag="hiU", name="hiU")
                            nc.scalar.copy(hiU[:], uC[:])
                            loU = cmtmp.tile([P, CH], BF16, tag="loU", name="loU")
                            nc.vector.tensor_tensor(
                                out=loU[:], in0=uC[:], in1=hiU[:],
                                op=mybir.AluOpType.subtract)
                            pt = psT.tile([P, CH // P, 2, P], BF16, space="PSUM")
                            pt2 = psT.tile([P, CH // P, 2, P], BF16, space="PSUM",
                                           name="pt2", tag="pt2")
                            idb = ident_q if qdt == BF16 else ident_p
                            for t in range(CH // P):
                                ts_ = slice(t * P, (t + 1) * P)
                                nc.tensor.transpose(pt[:, t, 0], eL[:, ts_],
                                                    ident_bfT[:])
                                nc.tensor.transpose(pt[:, t, 1], loL[:, ts_],
                                                    ident_bfT[:])
                                nc.tensor.transpose(pt2[:, t, 0], uC[:, ts_],
                                                    ident_bfT[:])
                                nc.tensor.transpose(pt2[:, t, 1], loU[:, ts_],
                                                    ident_bfT[:])
                            hsl = slice(k * 2 * CH, (k + 1) * 2 * CH)
                            nc.vector.tensor_copy(expLn[:, hsl], pt[:])
                            nc.vector.tensor_copy(Un[:, hsl], pt2[:])
                        else:
                            pt = psT.tile([P, CH // P, P], pdt, space="PSUM")
                            pt2 = psT.tile([P, CH // P, P], qdt, space="PSUM",
                                           name="pt2", tag="pt2")
                            for t in range(CH // P):
                                nc.tensor.transpose(
                                    pt[:, t], eL[:, t * P:(t + 1) * P], ident_p[:])
                                nc.tensor.transpose(
                                    pt2[:, t], uC[:, t * P:(t + 1) * P], ident_q[:])
                            nc.vector.tensor_copy(expLn[:, sl], pt[:])
                            nc.vector.tensor_copy(Un[:, sl], pt2[:])

                    # SpMM + per-chunk update
                    for g in range(cfg.ngroups):
                        Pt = psA.tile([P, G * CH], F32, space="PSUM", tag="p")
                        Qt = psA.tile([P, G * CH], F32, space="PSUM", tag="q")
                        JP = JG // 2    # dst-tile pairs per DMA slab
                        for jg in range(nt // JG):
                            at = adjp.tile([P, JP, 2, G * CH], FP8)
                            nc.sync.dma_start(
                                at[:], adj[g, :, jg * JP:(jg + 1) * JP, :, :])
                            for jl in range(JP):
                                jp = jg * JP + jl
                                st = (jp == 0)
                                sp = (jp == nt // 2 - 1)
                                if cfg.hl_fp8:
                                    for h in range(2):
                                        j = jp * 2 + h
                                        for sta, out in ((expLn, Pt), (Un, Qt)):
                                            lhs3 = sta[:, j * 2 * P:(j + 1) * 2 * P] \
                                                .rearrange("p (two c) -> p two c",
                                                           two=2)
                                            for i in range(G):
                                                rhs3 = at[:, jl, h,
                                                          i * CH:(i + 1) * CH] \
                                                    .rearrange(
                                                        "p (one s) -> p one s",
                                                        one=1) \
                                                    .to_broadcast([P, 2, CH])
                                                nc.tensor.matmul(
                                                    out[:, i * CH:(i + 1) * CH],
                                                    lhs3, rhs3,
                                                    start=st and h == 0,
                                                    stop=sp and h == 1,
                                                    perf_mode=mybir
                                                    .MatmulPerfMode.DoubleRow)
                                    continue
                                for sta, out, fp8m in ((expLn, Pt, cfg.fp8_p),
                                                       (Un, Qt, cfg.fp8_q)):
                                    if fp8m:
                                        lhs3 = sta[:, jp * 2 * P:(jp + 1) * 2 * P] \
                                            .rearrange("p (two c) -> p two c", two=2)
                                        for i in range(G):
                                            nc.tensor.matmul(
                                                out[:, i * CH:(i + 1) * CH],
                                                lhs3,
                                                at[:, jl, :, i * CH:(i + 1) * CH],
                                                start=st, stop=sp,
                                                perf_mode=mybir.MatmulPerfMode.DoubleRow)
                                    else:
                                        for h in range(2):
                                            j = jp * 2 + h
                                            for i in range(G):
                                                nc.tensor.matmul(
                                                    out[:, i * CH:(i + 1) * CH],
                                                    sta[:, j * P:(j + 1) * P],
                                                    at[:, jl, h, i * CH:(i + 1) * CH],
                                                    start=st and h == 0,
                                                    stop=sp and h == 1)
                        norms, lgfs = [], []
                        for i in range(G):
                            k = g * G + i
                            sl = slice(k * CH, (k + 1) * CH)
                            ii = slice(i * CH, (i + 1) * CH)
                            norm = upd.tile([P, CH], F32, tag=f"norm{i}",
                                            name=f"norm{i}")
                            nc.vector.tensor_mul(norm[:], expA_own[:, sl], Pt[:, ii])
                            lgf = upd.tile([P, CH], F32, tag=f"lgf{i}",
                                           name=f"lgf{i}")
                            nc.vector.tensor_mul(lgf[:], expA_own[:, sl], Qt[:, ii])
                            norms.append(norm)
                            lgfs.append(lgf)
                        for i in range(G):
                            k = g * G + i
                            sl = slice(k * CH, (k + 1) * CH)
                            norm, lgf = norms[i], lgfs[i]
                            nc.vector.tensor_scalar_add(norm[:], norm[:], 1.0)
                            r = upd.tile([P, CH], F32, tag="r")
                            nc.vector.reciprocal_approx_fast(r[:], norm[:])
                            lg = upd.tile([P, CH],
                                          BF16 if cfg.node_bf16 else F32,
                                          tag="lg")
                            nc.vector.tensor_mul(lg[:], lgf[:], r[:])
                            ps = psB.tile([P, CH], F32, space="PSUM")
                            nc.tensor.matmul(
                                ps[:], wl_t[:] if cfg.node_bf16 else wl_f[:],
                                lg[:])
                            sv = upd.tile([P, CH], F32, tag="sv")
                            nc.vector.tensor_mul(sv[:], states_cm[:, sl], r[:])
                            nc.vector.tensor_add(sv[:], sv[:], ps[:])
                            snew = snewp.tile([P, CH], F32)
                            nc.scalar.activation(snew[:], sv[:], Tanh,
                                                 bias=stb_t[:, :1])
                            if last:
                                nc.sync.dma_start(out_states[:, sl], snew[:])
                            else:
                                nc.vector.tensor_copy(states_cm[:, sl], snew[:])
                                nc.vector.tensor_copy(states_bf[:, sl], snew[:])
                                nc.sync.dma_start(inb[:, sl], snew[:])
                    # exchange: AllReduce-ADD own halves, peer = sum - own
                    if not last:
                        if no_collective:
                            nc.sync.dma_start(outb[:], inb[:])
                        else:
                            nc.gpsimd.collective_compute(
                                "AllReduce", mybir.AluOpType.add,
                                replica_groups=groups,
                                ins=[inb.opt()], outs=[outb.opt()])
                        gb = half // 4
                        for q in range(4):
                            qs = slice(q * gb, (q + 1) * gb)
                            sm = sump.tile([P, gb], F32)
                            nc.sync.dma_start(sm[:], outb[:, qs])
                            nc.vector.tensor_tensor(
                                out=states_cm[:, half + q * gb:half + (q + 1) * gb],
                                in0=sm[:], in1=states_cm[:, qs],
                                op=mybir.AluOpType.subtract)
                            nc.vector.tensor_copy(
                                states_bf[:, half + q * gb:half + (q + 1) * gb],
                                states_cm[:, half + q * gb:half + (q + 1) * gb])

    nc.compile()
    return nc


def preprocess(inputs, cfg: Cfg):
    """FULL numpy inputs -> per-core in_maps (index/layout work only)."""
    npad, half = cfg.npad, cfg.half
    fp8_np = mybir.dt.np(FP8)
    conn = np.asarray(inputs["connections"])
    objs = np.asarray(inputs["objects"], dtype=np.float32)
    nreal = objs.shape[1]
    bf = mybir.dt.np(BF16)
    w_sa_f32 = np.asarray(inputs["state_attention_W"], np.float32)
    w_l_f32 = np.asarray(inputs["linked_state_W"], np.float32)
    w_lsa_f32 = np.asarray(inputs["linked_state_attention_W"], np.float32)
    w_os = np.asarray(inputs["object_state_W"], np.float32)
    w_sa = np.asarray(inputs["state_attention_W"], np.float32).astype(bf)
    w_lsa = np.asarray(inputs["linked_state_attention_W"], np.float32).astype(bf)
    w_l = np.asarray(inputs["linked_state_W"], np.float32).astype(bf)
    att_b = np.asarray(inputs["attention_b"], np.float32).reshape(C, 1)
    st_b = np.asarray(inputs["state_b"], np.float32).reshape(C, 1)

    in_maps = []
    for b in range(B):
        a, d = conn[b, :, 0].astype(np.int64), conn[b, :, 1].astype(np.int64)
        adjm = np.zeros((npad, npad), np.uint8)
        np.add.at(adjm, (a, d), 1)
        np.add.at(adjm, (d, a), 1)
        objT = np.zeros((F, npad), np.float32)
        objT[:, :nreal] = objs[b].T
        for hi in range(2):
            off = hi * half
            oth = (1 - hi) * half
            # core-local dst (row) order: [own half rows || peer half rows];
            # src (column) range: own half
            sub = np.concatenate(
                [adjm[off:off + half, off:off + half],
                 adjm[oth:oth + half, off:off + half]], axis=0)
            sub = sub.reshape(cfg.nt // 2, 2, P, cfg.ngroups, cfg.group * CH)
            adj_in = np.ascontiguousarray(
                sub.transpose(3, 2, 0, 1, 4)).astype(fp8_np)    # [g, d, jp, 2, s]
            objT_local = np.concatenate(
                [objT[:, off:off + half], objT[:, oth:oth + half]], axis=1)
            in_maps.append({
                "adj": adj_in,
                "objT_full": np.ascontiguousarray(objT_local),
                "w_os": w_os, "w_sa": w_sa, "w_lsa": w_lsa, "w_l": w_l,
                "w_sa_f": w_sa_f32, "w_lsa_f": w_lsa_f32, "w_l_f": w_l_f32,
                "att_b": att_b, "st_b": st_b,
            })
    return in_maps


def postprocess(results, cfg: Cfg, nreal=N):
    out = np.zeros((B, nreal, C), np.float32)
    for b in range(B):
        full = np.concatenate(
            [results[2 * b]["out_states"], results[2 * b + 1]["out_states"]],
            axis=1)
        out[b] = full[:, :nreal].T
    return out


_CACHE = {}


def kernel(**inputs) -> np.ndarray:
    cfg = Cfg(npad=10240)
    if "nc" not in _CACHE:
        _CACHE["nc"] = build_kernel(cfg)
    nc = _CACHE["nc"]
    in_maps = preprocess(inputs, cfg)
    res = run_bass_kernel_spmd(nc, in_maps, core_ids=list(range(8)))
    return postprocess(res.results, cfg)


if __name__ == "__main__":
    import reference as R
    inputs = {k: np.asarray(v) for k, v in R.setup_inputs().items()}
    got = kernel(**inputs)
    exp = np.asarray(R.reference(**R.setup_inputs()))
    am = np.abs(exp).max()
    err = np.abs(got - exp).max()
    print(f"maxabs={am:.4f} err={err:.3e} rel={err / am:.3e}")
